# revision 22
# baseline (speedup 1.0000x reference)
"""GCNConv mean-aggregation kernel for 8 Trainium2 NeuronCores.

Reference computation:
    msgs   = x[src]                       # [E, D] gather
    summed = segment_sum(msgs, dst, N)    # [N, D]
    deg    = segment_sum(ones, dst, N)    # [N]
    h      = summed / max(deg, 1)
    out    = h @ W.T + b                  # [N, D_OUT]

Strategy (v2 — SWDGE gather tuned from HW microbenchmarks):
  - Shard edges by contiguous dst ranges: core c owns nodes
    [c*6272, (c+1)*6272).  Each core fully reduces its own node range.
  - Edges are grouped into 128-node dst windows (49 per core).  For each
    128-edge subtile we gather x[src] rows from HBM with dma_gather
    (SWDGE).  Rows are 256 B: 64 feats in bf16 + 64 zero pad.  bf16
    halves DMA-ring drain vs f32 rows and runs the PE at full bf16
    rate; the rel-err budget (2e-2) dwarfs bf16 quantization (~4e-3).
  - Gather calls are 32 subtiles (4096 descriptors) rotated over the 4
    SWDGE queues with single_packet=False.  Measured SWDGE descgen is
    ~2.2-2.5 ns/descriptor serialized on GpSimd — the kernel's
    critical path — so descriptor count is minimized: 128-node windows
    (vs 64) cut subtile padding, and degree is NOT computed on device
    (no weight column; the host precomputes 1/max(deg,1) from dst
    alone, pure index preprocessing).
  - A [128e, 128n] bf16 one-hot per subtile (DVE is_equal against an
    iota, 2x DVE rate in 16-bit) feeds  onehot.T @ msgs  into a
    [128, 64] PSUM f32 accumulation chain per window.
  - Normalize with the uploaded per-node reciprocals, PE-transpose
    [128, 64] -> [64, 128], apply W (lhsT = W.T) and bias, and write
    out.T slices ([64, 6272] per core).  Host reassembles/transposes.
  - dma_gather indices are int16, so x is staged into two gather tables
    (src < 32767 and src >= 32767), each with a zero row at index 0
    used by padding edges (contributes 0 to the window sums).
"""

import sys

sys.path.insert(0, "/opt/trn_rl_repo")

import ml_dtypes
import numpy as np

import concourse.bacc as bacc
import concourse.mybir as mybir
import concourse.tile as tile
from concourse.bass_utils import run_bass_kernel_spmd

N_NODES = 50000
N_EDGES = 800000
D = 64
N_CORES = 8
NPC = 6272          # nodes per core (= 49 windows of 128)
WIN = 128           # dst-window width per PSUM accumulation group
N_WIN = NPC // WIN  # 49
SPLIT = 32767       # src < SPLIT -> lo table, else hi table
ROW = 128           # gather row: 64 bf16 feats + 64 bf16 zero pad (256 B)
CHUNK = 32          # subtiles (of 128 edges) per dma_gather call
NQ = 4              # SWDGE queues for parallel gather descriptor work

F32 = mybir.dt.float32
BF16 = mybir.dt.bfloat16
I16 = mybir.dt.int16
BFNP = ml_dtypes.bfloat16

# Results of the most recent run (for test harness inspection).
LAST = {}


def _prep(x, src, dst):
    """Host-side sharding: build bf16 gather tables, per-core padded edge
    streams (int16 gather idx + bf16 dst-rel), per-core reciprocal
    degrees, and per-window subtile budgets (shared across cores; SPMD
    program structure)."""
    x = np.asarray(x, dtype=np.float32)
    src = np.asarray(src, dtype=np.int64)
    dst = np.asarray(dst, dtype=np.int64)

    n_lo = SPLIT
    n_hi = N_NODES - SPLIT
    xlo = np.zeros((n_lo + 1, ROW), dtype=BFNP)
    xlo[1:, :D] = x[:SPLIT].astype(BFNP)
    xhi = np.zeros((n_hi + 1, ROW), dtype=BFNP)
    xhi[1:, :D] = x[SPLIT:].astype(BFNP)

    cls = (src >= SPLIT).astype(np.int64)

    # --- balance nodes across the 392 (core, window) bins ---------------
    # Subtile padding is ceil(max-over-cores/128) per (window, class); an
    # LPT assignment of nodes (weighted by per-class in-degree) makes the
    # per-bin counts nearly equal, collapsing the padding.  The host owns
    # the node->(core, window, lane) permutation and reassembles at the
    # end, so the device never sees node ids.
    import heapq

    a_deg = np.bincount(dst[cls == 0], minlength=N_NODES)
    b_deg = np.bincount(dst[cls == 1], minlength=N_NODES)
    tot_deg = a_deg + b_deg
    n_bins = N_CORES * N_WIN
    order_n = np.argsort(-tot_deg, kind="stable")
    heap = [(0.0, b) for b in range(n_bins)]
    heapq.heapify(heap)
    bin_fill = np.zeros(n_bins, dtype=np.int64)
    bin_a = np.zeros(n_bins, dtype=np.int64)
    bin_b = np.zeros(n_bins, dtype=np.int64)
    node_bin = np.empty(N_NODES, dtype=np.int64)
    node_lane = np.empty(N_NODES, dtype=np.int64)
    wa = 1.0 / max(1, a_deg.sum() // n_bins)
    wb = 1.0 / max(1, b_deg.sum() // n_bins)
    for v in order_n:
        while True:
            load, bn = heapq.heappop(heap)
            if bin_fill[bn] < WIN:
                break
        node_bin[v] = bn
        node_lane[v] = bin_fill[bn]
        bin_fill[bn] += 1
        bin_a[bn] += a_deg[v]
        bin_b[bn] += b_deg[v]
        if bin_fill[bn] < WIN:
            heapq.heappush(heap, (bin_a[bn] * wa + bin_b[bn] * wb, bn))

    deg = np.maximum(tot_deg, 1).astype(np.float32)
    recip_pc = np.ones((N_CORES, WIN, N_WIN), dtype=np.float32)
    for c in range(N_CORES):
        for w in range(N_WIN):
            bn = c * N_WIN + w
            sel = node_bin == bn
            recip_pc[c, node_lane[sel], w] = 1.0 / deg[sel]
    binnode = np.full((n_bins, WIN), -1, dtype=np.int64)
    binnode[node_bin, node_lane] = np.arange(N_NODES)

    key = node_bin[dst] * 2 + cls
    order = np.argsort(key, kind="stable")
    src_s, dst_s = src[order], dst[order]
    lane_s = node_lane[dst_s]

    n_groups = (N_CORES * N_WIN) * 2
    counts = np.bincount(key[order], minlength=n_groups)
    starts = np.zeros(n_groups + 1, dtype=np.int64)
    np.cumsum(counts, out=starts[1:])

    cnt = counts.reshape(N_CORES, N_WIN, 2)
    kA = np.maximum(1, -(-cnt[:, :, 0].max(axis=0) // 128))  # [N_WIN]
    kB = -(-cnt[:, :, 1].max(axis=0) // 128)                  # [N_WIN]
    SA = int(kA.sum())
    SB = int(kB.sum())

    idx_lo = (src_s + 1).astype(np.int16)
    idx_hi = (src_s - SPLIT + 1).astype(np.int16)

    offA = np.zeros(N_WIN + 1, dtype=np.int64)
    np.cumsum(kA, out=offA[1:])
    offB = np.zeros(N_WIN + 1, dtype=np.int64)
    np.cumsum(kB, out=offB[1:])

    per_core = []
    for c in range(N_CORES):
        # pad lanes: idx -1 (SWDGE skips the descriptor entirely — free)
        # and dst_rel 200 (outside iota range, so the one-hot column is
        # all-zero and the stale SBUF lane never reaches the PSUM sums).
        iA = np.zeros(SA * 128, dtype=np.int16)
        dA = np.full(SA * 128, 200.0, dtype=BFNP)
        iB = np.zeros(max(SB, 1) * 128, dtype=np.int16)
        dB = np.full(max(SB, 1) * 128, 200.0, dtype=BFNP)
        for w in range(N_WIN):
            g = (c * N_WIN + w) * 2
            s0, s1 = starts[g], starts[g + 1]
            p0 = int(offA[w]) * 128
            iA[p0 : p0 + (s1 - s0)] = idx_lo[s0:s1]
            dA[p0 : p0 + (s1 - s0)] = lane_s[s0:s1].astype(BFNP)
            s0, s1 = starts[g + 1], starts[g + 2]
            p0 = int(offB[w]) * 128
            iB[p0 : p0 + (s1 - s0)] = idx_hi[s0:s1]
            dB[p0 : p0 + (s1 - s0)] = lane_s[s0:s1].astype(BFNP)
        per_core.append((iA, dA, iB, dB))

    return xlo, xhi, kA, kB, SA, SB, offA, offB, per_core, recip_pc, binnode


def _wrap_idx(idx_flat):
    """int16 stream -> dma_gather layout [128, n/16]: value i at
    [i % 16, i // 16], replicated across the 8 groups of 16 partitions."""
    a = idx_flat.reshape(-1, 16).T
    return np.tile(a, (8, 1)).copy()


def _wrap_dst(d_flat):
    """f32 stream -> [128, S]: subtile s lane e at [e, s]."""
    return np.ascontiguousarray(d_flat.reshape(-1, 128).T)


def _build_program(kA, kB, SA, SB, offA, offB):
    nc = bacc.Bacc(
        "TRN2", target_bir_lowering=False, debug=False, num_swdge_queues=NQ
    )

    t_xlo = nc.dram_tensor("xlo", [SPLIT + 1, ROW], BF16, kind="ExternalInput")
    t_xhi = nc.dram_tensor(
        "xhi", [N_NODES - SPLIT + 1, ROW], BF16, kind="ExternalInput"
    )
    t_wt = nc.dram_tensor("wt", [D, D], F32, kind="ExternalInput")
    t_b = nc.dram_tensor("bias", [D, 1], F32, kind="ExternalInput")
    t_rc = nc.dram_tensor("recip", [WIN, N_WIN], F32, kind="ExternalInput")
    t_ia = nc.dram_tensor("idxa", [128, SA * 8], I16, kind="ExternalInput")
    t_da = nc.dram_tensor("dsta", [128, SA], BF16, kind="ExternalInput")
    SBp = max(SB, 1)
    t_ib = nc.dram_tensor("idxb", [128, SBp * 8], I16, kind="ExternalInput")
    t_db = nc.dram_tensor("dstb", [128, SBp], BF16, kind="ExternalInput")
    t_iota = nc.dram_tensor("iota", [128, CHUNK * WIN], BF16, kind="ExternalInput")
    t_id = nc.dram_tensor("ident", [128, 128], F32, kind="ExternalInput")
    t_out = nc.dram_tensor("out", [D, NPC], F32, kind="ExternalOutput")

    def _calls(S):
        calls = [(p, min(CHUNK, S - p)) for p in range(0, S, CHUNK)]
        if calls and calls[-1][1] > CHUNK // 2:
            p, n = calls.pop()
            calls += [(p, n - n // 2), (p + n - n // 2, n // 2)]
        return calls

    callsA = _calls(SA)
    callsB = _calls(SB)

    with tile.TileContext(nc) as tc:
        with (
            tc.tile_pool(name="const", bufs=1) as cpool,
            tc.tile_pool(name="idx", bufs=1) as ipool,
            tc.tile_pool(name="msgsa", bufs=4) as mpa,
            tc.tile_pool(name="msgsb", bufs=3) as mpb,
            tc.tile_pool(name="oha", bufs=4) as opa,
            tc.tile_pool(name="ohb", bufs=3) as opb,
            tc.tile_pool(name="norm", bufs=6) as npool,
            tc.tile_pool(name="hpo", bufs=4) as hpool,
            tc.tile_pool(name="psacc", bufs=4, space="PSUM") as ps_acc,
            tc.tile_pool(name="pstr", bufs=2, space="PSUM") as ps_tr,
            tc.tile_pool(name="psz", bufs=2, space="PSUM") as ps_z,
        ):
            ident = cpool.tile([128, 128], F32)
            nc.sync.dma_start(out=ident[:], in_=t_id[:])
            wt_sb = cpool.tile([D, D], F32)
            nc.sync.dma_start(out=wt_sb[:], in_=t_wt[:])
            b_sb = cpool.tile([D, 1], F32)
            nc.sync.dma_start(out=b_sb[:], in_=t_b[:])
            rc_sb = cpool.tile([WIN, N_WIN], F32)
            nc.sync.dma_start(out=rc_sb[:], in_=t_rc[:])
            iota_f = cpool.tile([128, CHUNK * WIN], BF16)
            nc.sync.dma_start(out=iota_f[:], in_=t_iota[:])

            ia_sb = ipool.tile([128, SA * 8], I16)
            nc.sync.dma_start(out=ia_sb[:], in_=t_ia[:])
            da_sb = ipool.tile([128, SA], BF16)
            nc.sync.dma_start(out=da_sb[:], in_=t_da[:])
            ib_sb = ipool.tile([128, SBp * 8], I16)
            nc.sync.dma_start(out=ib_sb[:], in_=t_ib[:])
            db_sb = ipool.tile([128, SBp], BF16)
            nc.sync.dma_start(out=db_sb[:], in_=t_db[:])

            out_sb = cpool.tile([D, NPC], F32)

            # warm the 4 SWDGE queues with tiny gathers (zero row) so the
            # first real calls don't pay queue-init
            widx = cpool.tile([128, 8], I16)
            nc.scalar.memzero(widx[:])
            wdst = cpool.tile([128, 1, ROW], BF16)
            for q in range(NQ):
                nc.gpsimd.dma_gather(
                    wdst[:],
                    t_xlo[:],
                    widx[:],
                    128,
                    128,
                    ROW,
                    single_packet=False,
                    queue_num=q,
                )

            chunk_tiles = {0: [], 1: []}
            call_no = [0]

            def emit_chunk(st, k):
                if st == 0:
                    pos, nsub = callsA[k]
                    mp, op, tsrc, isb, dsb = mpa, opa, t_xlo, ia_sb, da_sb
                else:
                    pos, nsub = callsB[k]
                    mp, op, tsrc, isb, dsb = mpb, opb, t_xhi, ib_sb, db_sb
                msgs = mp.tile([128, CHUNK, ROW], BF16)
                nidx = nsub * 128
                # single_packet=False: one packet per descriptor (the default
                # single-packet mode wedges the SDMA engine beyond ~64
                # descs/lane).  Rotating queue_num spreads ring drain over
                # the 4 SWDGE queues.
                nc.gpsimd.dma_gather(
                    msgs[:, :nsub, :],
                    tsrc[:],
                    isb[:, pos * 8 : pos * 8 + nsub * 8],
                    nidx,
                    nidx,
                    ROW,
                    single_packet=False,
                    queue_num=call_no[0] % NQ,
                )
                call_no[0] += 1
                # chunked one-hot: f32 iota vs broadcast f32 dst-rel, bf16
                # out for the full-rate bf16 matmul.  (Per-subtile
                # tensor_scalar variants lose: ~578 ns fixed cost per DVE
                # instruction.)
                oh = op.tile([128, CHUNK * WIN], BF16)
                dst_b = (
                    dsb[:, pos : pos + nsub]
                    .unsqueeze(2)
                    .to_broadcast([128, nsub, WIN])
                )
                nc.vector.tensor_tensor(
                    out=oh[:, : nsub * WIN].rearrange("p (s w) -> p s w", w=WIN),
                    in0=iota_f[:, : nsub * WIN].rearrange(
                        "p (s w) -> p s w", w=WIN
                    ),
                    in1=dst_b,
                    op=mybir.AluOpType.is_equal,
                )
                chunk_tiles[st].append((msgs, oh))

            cursor = [0, 0]
            call_of = {}
            for st, calls in ((0, callsA), (1, callsB)):
                for k, (pos, nsub) in enumerate(calls):
                    for s in range(pos, pos + nsub):
                        call_of[(st, s)] = (k, s - pos)

            def tiles_for(st, s):
                k, col = call_of[(st, s)]
                while cursor[st] <= k:
                    emit_chunk(st, cursor[st])
                    cursor[st] += 1
                msgs, oh = chunk_tiles[st][k]
                return msgs, oh, col

            for w in range(N_WIN):
                subs = [(0, int(offA[w]) + j) for j in range(int(kA[w]))]
                subs += [(1, int(offB[w]) + j) for j in range(int(kB[w]))]
                ps = ps_acc.tile([WIN, D], F32)
                for j, (st, s) in enumerate(subs):
                    msgs, oh, col = tiles_for(st, s)
                    nc.tensor.matmul(
                        out=ps[:],
                        lhsT=oh[:, col * WIN : (col + 1) * WIN],
                        rhs=msgs[:, col, :D],
                        start=(j == 0),
                        stop=(j == len(subs) - 1),
                    )
                h_w = npool.tile([WIN, D], F32)
                nc.vector.tensor_scalar_mul(h_w[:], ps[:], rc_sb[:, w : w + 1])
                pst = ps_tr.tile([D, WIN], F32)
                nc.tensor.transpose(out=pst[:], in_=h_w[:], identity=ident[:])
                ht = hpool.tile([D, WIN], F32)
                nc.vector.tensor_copy(out=ht[:], in_=pst[:])
                z = ps_z.tile([D, WIN], F32)
                nc.tensor.matmul(
                    out=z[:], lhsT=wt_sb[:], rhs=ht[:], start=True, stop=True
                )
                nc.vector.tensor_scalar_add(
                    out_sb[:, w * WIN : (w + 1) * WIN], z[:], b_sb[:]
                )
                if (w + 1) % 8 == 0 or w == N_WIN - 1:
                    w0 = (w // 8) * 8
                    nc.sync.dma_start(
                        out=t_out[:, w0 * WIN : (w + 1) * WIN],
                        in_=out_sb[:, w0 * WIN : (w + 1) * WIN],
                    )

    nc.compile()
    return nc


def kernel(x, src, dst, W, b):
    x = np.asarray(x, dtype=np.float32)
    W = np.asarray(W, dtype=np.float32)
    b = np.asarray(b, dtype=np.float32)

    (xlo, xhi, kA, kB, SA, SB, offA, offB, per_core, recip_pc,
     binnode) = _prep(x, src, dst)
    print(f"kernel: SA={SA} SB={SB} descs={(SA+SB)*128}", file=sys.stderr)
    nc = _build_program(kA, kB, SA, SB, offA, offB)

    wt = np.ascontiguousarray(W.T)
    bcol = np.ascontiguousarray(b.reshape(D, 1))
    iota_arr = np.tile(
        np.arange(WIN, dtype=np.float32)[None, :], (128, CHUNK)
    ).astype(BFNP)
    ident_arr = np.eye(128, dtype=np.float32)

    in_maps = []
    for c in range(N_CORES):
        iA, dA, iB, dB = per_core[c]
        in_maps.append(
            {
                "xlo": xlo,
                "xhi": xhi,
                "wt": wt,
                "bias": bcol,
                "recip": np.ascontiguousarray(recip_pc[c]),
                "idxa": _wrap_idx(iA),
                "dsta": _wrap_dst(dA),
                "idxb": _wrap_idx(iB),
                "dstb": _wrap_dst(dB),
                "iota": iota_arr,
                "ident": ident_arr,
            }
        )

    res = run_bass_kernel_spmd(nc, in_maps, list(range(N_CORES)))
    LAST["results"] = res
    LAST["exec_time_ns"] = res.exec_time_ns

    out_t = np.concatenate([res.results[c]["out"] for c in range(N_CORES)], axis=1)
    cols = out_t.T  # [N_CORES*NPC, 64]: column (c,w,lane) at c*NPC + w*WIN + lane
    result = np.empty((N_NODES, D), dtype=np.float32)
    flat_nodes = binnode.reshape(N_CORES, N_WIN, WIN)
    for c in range(N_CORES):
        for w in range(N_WIN):
            nodes = flat_nodes[c, w]
            valid = nodes >= 0
            result[nodes[valid]] = cols[
                c * NPC + w * WIN : c * NPC + (w + 1) * WIN
            ][valid]
    return result


# revision 24
# speedup vs baseline: 1.0693x; 1.0693x over previous
"""GCNConv mean-aggregation kernel for 8 Trainium2 NeuronCores.

Reference computation:
    msgs   = x[src]                       # [E, D] gather
    summed = segment_sum(msgs, dst, N)    # [N, D]
    deg    = segment_sum(ones, dst, N)    # [N]
    h      = summed / max(deg, 1)
    out    = h @ W.T + b                  # [N, D_OUT]

Strategy (v2 — SWDGE gather tuned from HW microbenchmarks):
  - Shard edges by contiguous dst ranges: core c owns nodes
    [c*6272, (c+1)*6272).  Each core fully reduces its own node range.
  - Edges are grouped into 128-node dst windows (49 per core).  For each
    128-edge subtile we gather x[src] rows from HBM with dma_gather
    (SWDGE).  Rows are 256 B: 64 feats in bf16 + 64 zero pad.  bf16
    halves DMA-ring drain vs f32 rows and runs the PE at full bf16
    rate; the rel-err budget (2e-2) dwarfs bf16 quantization (~4e-3).
  - Gather calls are 32 subtiles (4096 descriptors) rotated over the 4
    SWDGE queues with single_packet=False.  Measured SWDGE descgen is
    ~2.2-2.5 ns/descriptor serialized on GpSimd — the kernel's
    critical path — so descriptor count is minimized: 128-node windows
    (vs 64) cut subtile padding, and degree is NOT computed on device
    (no weight column; the host precomputes 1/max(deg,1) from dst
    alone, pure index preprocessing).
  - A [128e, 128n] bf16 one-hot per subtile (DVE is_equal against an
    iota, 2x DVE rate in 16-bit) feeds  onehot.T @ msgs  into a
    [128, 64] PSUM f32 accumulation chain per window.
  - Normalize with the uploaded per-node reciprocals, PE-transpose
    [128, 64] -> [64, 128], apply W (lhsT = W.T) and bias, and write
    out.T slices ([64, 6272] per core).  Host reassembles/transposes.
  - dma_gather indices are int16, so x is staged into two gather tables
    (src < 32767 and src >= 32767), each with a zero row at index 0
    used by padding edges (contributes 0 to the window sums).
"""

import sys

sys.path.insert(0, "/opt/trn_rl_repo")

import ml_dtypes
import numpy as np

import concourse.bacc as bacc
import concourse.mybir as mybir
import concourse.tile as tile
from concourse.bass_utils import run_bass_kernel_spmd

N_NODES = 50000
N_EDGES = 800000
D = 64
N_CORES = 8
NPC = 6272          # nodes per core (= 49 windows of 128)
WIN = 128           # dst-window width per PSUM accumulation group
N_WIN = NPC // WIN  # 49
SPLIT = 32767       # src < SPLIT -> lo table, else hi table
ROW = 128           # gather row: 64 bf16 feats + 64 bf16 zero pad (256 B)
CHUNK = 32          # subtiles (of 128 edges) per dma_gather call
NQ = 4              # SWDGE queues for parallel gather descriptor work

F32 = mybir.dt.float32
BF16 = mybir.dt.bfloat16
I16 = mybir.dt.int16
BFNP = ml_dtypes.bfloat16

# Results of the most recent run (for test harness inspection).
LAST = {}


def _prep(x, src, dst):
    """Host-side sharding: build bf16 gather tables, per-core padded edge
    streams (int16 gather idx + bf16 dst-rel), per-core reciprocal
    degrees, and per-window subtile budgets (shared across cores; SPMD
    program structure)."""
    x = np.asarray(x, dtype=np.float32)
    src = np.asarray(src, dtype=np.int64)
    dst = np.asarray(dst, dtype=np.int64)

    n_lo = SPLIT
    n_hi = N_NODES - SPLIT
    xlo = np.zeros((n_lo + 1, ROW), dtype=BFNP)
    xlo[1:, :D] = x[:SPLIT].astype(BFNP)
    xhi = np.zeros((n_hi + 1, ROW), dtype=BFNP)
    xhi[1:, :D] = x[SPLIT:].astype(BFNP)

    cls = (src >= SPLIT).astype(np.int64)

    # --- balance nodes across the 392 (core, window) bins ---------------
    # Subtile padding is ceil(max-over-cores/128) per (window, class); an
    # LPT assignment of nodes (weighted by per-class in-degree) makes the
    # per-bin counts nearly equal, collapsing the padding.  The host owns
    # the node->(core, window, lane) permutation and reassembles at the
    # end, so the device never sees node ids.
    import heapq

    a_deg = np.bincount(dst[cls == 0], minlength=N_NODES)
    b_deg = np.bincount(dst[cls == 1], minlength=N_NODES)
    tot_deg = a_deg + b_deg
    n_bins = N_CORES * N_WIN
    order_n = np.argsort(-tot_deg, kind="stable")
    heap = [(0.0, b) for b in range(n_bins)]
    heapq.heapify(heap)
    bin_fill = np.zeros(n_bins, dtype=np.int64)
    bin_a = np.zeros(n_bins, dtype=np.int64)
    bin_b = np.zeros(n_bins, dtype=np.int64)
    node_bin = np.empty(N_NODES, dtype=np.int64)
    node_lane = np.empty(N_NODES, dtype=np.int64)
    wa = 1.0 / max(1, a_deg.sum() // n_bins)
    wb = 1.0 / max(1, b_deg.sum() // n_bins)
    for v in order_n:
        while True:
            load, bn = heapq.heappop(heap)
            if bin_fill[bn] < WIN:
                break
        node_bin[v] = bn
        node_lane[v] = bin_fill[bn]
        bin_fill[bn] += 1
        bin_a[bn] += a_deg[v]
        bin_b[bn] += b_deg[v]
        if bin_fill[bn] < WIN:
            heapq.heappush(heap, (bin_a[bn] * wa + bin_b[bn] * wb, bn))

    deg = np.maximum(tot_deg, 1).astype(np.float32)
    recip_pc = np.ones((N_CORES, WIN, N_WIN), dtype=np.float32)
    for c in range(N_CORES):
        for w in range(N_WIN):
            bn = c * N_WIN + w
            sel = node_bin == bn
            recip_pc[c, node_lane[sel], w] = 1.0 / deg[sel]
    binnode = np.full((n_bins, WIN), -1, dtype=np.int64)
    binnode[node_bin, node_lane] = np.arange(N_NODES)

    key = node_bin[dst] * 2 + cls
    order = np.argsort(key, kind="stable")
    src_s, dst_s = src[order], dst[order]
    lane_s = node_lane[dst_s]

    n_groups = (N_CORES * N_WIN) * 2
    counts = np.bincount(key[order], minlength=n_groups)
    starts = np.zeros(n_groups + 1, dtype=np.int64)
    np.cumsum(counts, out=starts[1:])

    cnt = counts.reshape(N_CORES, N_WIN, 2)
    kA = np.maximum(1, -(-cnt[:, :, 0].max(axis=0) // 128))  # [N_WIN]
    kB = -(-cnt[:, :, 1].max(axis=0) // 128)                  # [N_WIN]
    SA = int(kA.sum())
    SB = int(kB.sum())

    idx_lo = (src_s + 1).astype(np.int16)
    idx_hi = (src_s - SPLIT + 1).astype(np.int16)

    offA = np.zeros(N_WIN + 1, dtype=np.int64)
    np.cumsum(kA, out=offA[1:])
    offB = np.zeros(N_WIN + 1, dtype=np.int64)
    np.cumsum(kB, out=offB[1:])

    per_core = []
    for c in range(N_CORES):
        # pad lanes: idx -1 (SWDGE skips the descriptor entirely — free)
        # and dst_rel 200 (outside iota range, so the one-hot column is
        # all-zero and the stale SBUF lane never reaches the PSUM sums).
        iA = np.zeros(SA * 128, dtype=np.int16)
        dA = np.full(SA * 128, 200.0, dtype=BFNP)
        iB = np.zeros(max(SB, 1) * 128, dtype=np.int16)
        dB = np.full(max(SB, 1) * 128, 200.0, dtype=BFNP)
        for w in range(N_WIN):
            g = (c * N_WIN + w) * 2
            s0, s1 = starts[g], starts[g + 1]
            p0 = int(offA[w]) * 128
            iA[p0 : p0 + (s1 - s0)] = idx_lo[s0:s1]
            dA[p0 : p0 + (s1 - s0)] = lane_s[s0:s1].astype(BFNP)
            s0, s1 = starts[g + 1], starts[g + 2]
            p0 = int(offB[w]) * 128
            iB[p0 : p0 + (s1 - s0)] = idx_hi[s0:s1]
            dB[p0 : p0 + (s1 - s0)] = lane_s[s0:s1].astype(BFNP)
        per_core.append((iA, dA, iB, dB))

    return xlo, xhi, kA, kB, SA, SB, offA, offB, per_core, recip_pc, binnode


def _wrap_idx(idx_flat):
    """int16 stream -> dma_gather layout [128, n/16]: value i at
    [i % 16, i // 16], replicated across the 8 groups of 16 partitions."""
    a = idx_flat.reshape(-1, 16).T
    return np.tile(a, (8, 1)).copy()


def _wrap_dst(d_flat):
    """f32 stream -> [128, S]: subtile s lane e at [e, s]."""
    return np.ascontiguousarray(d_flat.reshape(-1, 128).T)


def _build_program(kA, kB, SA, SB, offA, offB):
    nc = bacc.Bacc(
        "TRN2", target_bir_lowering=False, debug=False, num_swdge_queues=NQ
    )

    t_xlo = nc.dram_tensor("xlo", [SPLIT + 1, ROW], BF16, kind="ExternalInput")
    t_xhi = nc.dram_tensor(
        "xhi", [N_NODES - SPLIT + 1, ROW], BF16, kind="ExternalInput"
    )
    t_wt = nc.dram_tensor("wt", [D, D], F32, kind="ExternalInput")
    t_b = nc.dram_tensor("bias", [D, 1], F32, kind="ExternalInput")
    t_rc = nc.dram_tensor("recip", [WIN, N_WIN], F32, kind="ExternalInput")
    t_ia = nc.dram_tensor("idxa", [128, SA * 8], I16, kind="ExternalInput")
    t_da = nc.dram_tensor("dsta", [128, SA], BF16, kind="ExternalInput")
    SBp = max(SB, 1)
    t_ib = nc.dram_tensor("idxb", [128, SBp * 8], I16, kind="ExternalInput")
    t_db = nc.dram_tensor("dstb", [128, SBp], BF16, kind="ExternalInput")
    t_iota = nc.dram_tensor("iota", [128, WIN], BF16, kind="ExternalInput")
    t_id = nc.dram_tensor("ident", [128, 128], F32, kind="ExternalInput")
    t_out = nc.dram_tensor("out", [D, NPC], F32, kind="ExternalOutput")

    def _calls(S):
        calls = [(p, min(CHUNK, S - p)) for p in range(0, S, CHUNK)]
        if calls and calls[-1][1] > CHUNK // 2:
            p, n = calls.pop()
            calls += [(p, n - n // 2), (p + n - n // 2, n // 2)]
        return calls

    callsA = _calls(SA)
    callsB = _calls(SB)

    with tile.TileContext(nc) as tc:
        with (
            tc.tile_pool(name="const", bufs=1) as cpool,
            tc.tile_pool(name="idx", bufs=1) as ipool,
            tc.tile_pool(name="msgsa", bufs=4) as mpa,
            tc.tile_pool(name="msgsb", bufs=3) as mpb,
            tc.tile_pool(name="oha", bufs=4) as opa,
            tc.tile_pool(name="ohb", bufs=3) as opb,
            tc.tile_pool(name="norm", bufs=6) as npool,
            tc.tile_pool(name="hpo", bufs=4) as hpool,
            tc.tile_pool(name="psacc", bufs=4, space="PSUM") as ps_acc,
            tc.tile_pool(name="pstr", bufs=2, space="PSUM") as ps_tr,
            tc.tile_pool(name="psz", bufs=2, space="PSUM") as ps_z,
        ):
            ident = cpool.tile([128, 128], F32)
            nc.sync.dma_start(out=ident[:], in_=t_id[:])
            wt_sb = cpool.tile([D, D], F32)
            nc.sync.dma_start(out=wt_sb[:], in_=t_wt[:])
            b_sb = cpool.tile([D, 1], F32)
            nc.sync.dma_start(out=b_sb[:], in_=t_b[:])
            rc_sb = cpool.tile([WIN, N_WIN], F32)
            nc.sync.dma_start(out=rc_sb[:], in_=t_rc[:])
            iota_f = cpool.tile([128, WIN], BF16)
            nc.sync.dma_start(out=iota_f[:], in_=t_iota[:])

            ia_sb = ipool.tile([128, SA * 8], I16)
            da_sb = ipool.tile([128, SA], BF16)
            nc.sync.dma_start(out=da_sb[:], in_=t_da[:])
            ib_sb = ipool.tile([128, SBp * 8], I16)
            db_sb = ipool.tile([128, SBp], BF16)
            nc.sync.dma_start(out=db_sb[:], in_=t_db[:])

            out_sb = cpool.tile([D, NPC], F32)

            chunk_tiles = {0: [], 1: []}
            call_no = [0]

            def emit_chunk(st, k):
                if st == 0:
                    pos, nsub = callsA[k]
                    mp, op, tsrc, isb, dsb = mpa, opa, t_xlo, ia_sb, da_sb
                else:
                    pos, nsub = callsB[k]
                    mp, op, tsrc, isb, dsb = mpb, opb, t_xhi, ib_sb, db_sb
                # just-in-time idx slice load: spreads the 2.7 MB of index
                # tables across the run instead of jamming the DMA engines
                # (and gather-ring drains) at startup
                t_i = t_ia if st == 0 else t_ib
                nc.sync.dma_start(
                    out=isb[:, pos * 8 : pos * 8 + nsub * 8],
                    in_=t_i[:, pos * 8 : pos * 8 + nsub * 8],
                )
                msgs = mp.tile([128, CHUNK, ROW], BF16)
                nidx = nsub * 128
                # single_packet=False: one packet per descriptor (the default
                # single-packet mode wedges the SDMA engine beyond ~64
                # descs/lane).  Rotating queue_num spreads ring drain over
                # the 4 SWDGE queues.
                nc.gpsimd.dma_gather(
                    msgs[:, :nsub, :],
                    tsrc[:],
                    isb[:, pos * 8 : pos * 8 + nsub * 8],
                    nidx,
                    nidx,
                    ROW,
                    single_packet=False,
                    queue_num=call_no[0] % NQ,
                )
                call_no[0] += 1
                # chunked one-hot: f32 iota vs broadcast f32 dst-rel, bf16
                # out for the full-rate bf16 matmul.  (Per-subtile
                # tensor_scalar variants lose: ~578 ns fixed cost per DVE
                # instruction.)
                oh = op.tile([128, CHUNK * WIN], BF16)
                dst_b = (
                    dsb[:, pos : pos + nsub]
                    .unsqueeze(2)
                    .to_broadcast([128, nsub, WIN])
                )
                nc.vector.tensor_tensor(
                    out=oh[:, : nsub * WIN].rearrange("p (s w) -> p s w", w=WIN),
                    in0=iota_f[:].unsqueeze(1).to_broadcast([128, nsub, WIN]),
                    in1=dst_b,
                    op=mybir.AluOpType.is_equal,
                )
                chunk_tiles[st].append((msgs, oh))

            cursor = [0, 0]
            call_of = {}
            for st, calls in ((0, callsA), (1, callsB)):
                for k, (pos, nsub) in enumerate(calls):
                    for s in range(pos, pos + nsub):
                        call_of[(st, s)] = (k, s - pos)

            def tiles_for(st, s):
                k, col = call_of[(st, s)]
                while cursor[st] <= k:
                    emit_chunk(st, cursor[st])
                    cursor[st] += 1
                msgs, oh = chunk_tiles[st][k]
                return msgs, oh, col

            for w in range(N_WIN):
                subs = [(0, int(offA[w]) + j) for j in range(int(kA[w]))]
                subs += [(1, int(offB[w]) + j) for j in range(int(kB[w]))]
                ps = ps_acc.tile([WIN, D], F32)
                for j, (st, s) in enumerate(subs):
                    msgs, oh, col = tiles_for(st, s)
                    nc.tensor.matmul(
                        out=ps[:],
                        lhsT=oh[:, col * WIN : (col + 1) * WIN],
                        rhs=msgs[:, col, :D],
                        start=(j == 0),
                        stop=(j == len(subs) - 1),
                    )
                h_w = npool.tile([WIN, D], F32)
                nc.vector.tensor_scalar_mul(h_w[:], ps[:], rc_sb[:, w : w + 1])
                pst = ps_tr.tile([D, WIN], F32)
                nc.tensor.transpose(out=pst[:], in_=h_w[:], identity=ident[:])
                ht = hpool.tile([D, WIN], F32)
                nc.vector.tensor_copy(out=ht[:], in_=pst[:])
                z = ps_z.tile([D, WIN], F32)
                nc.tensor.matmul(
                    out=z[:], lhsT=wt_sb[:], rhs=ht[:], start=True, stop=True
                )
                nc.vector.tensor_scalar_add(
                    out_sb[:, w * WIN : (w + 1) * WIN], z[:], b_sb[:]
                )
                if (w + 1) % 8 == 0 or w == N_WIN - 1:
                    w0 = (w // 8) * 8
                    nc.sync.dma_start(
                        out=t_out[:, w0 * WIN : (w + 1) * WIN],
                        in_=out_sb[:, w0 * WIN : (w + 1) * WIN],
                    )

    nc.compile()
    return nc


def kernel(x, src, dst, W, b):
    x = np.asarray(x, dtype=np.float32)
    W = np.asarray(W, dtype=np.float32)
    b = np.asarray(b, dtype=np.float32)

    (xlo, xhi, kA, kB, SA, SB, offA, offB, per_core, recip_pc,
     binnode) = _prep(x, src, dst)
    print(f"kernel: SA={SA} SB={SB} descs={(SA+SB)*128}", file=sys.stderr)
    nc = _build_program(kA, kB, SA, SB, offA, offB)

    wt = np.ascontiguousarray(W.T)
    bcol = np.ascontiguousarray(b.reshape(D, 1))
    iota_arr = np.tile(
        np.arange(WIN, dtype=np.float32)[None, :], (128, 1)
    ).astype(BFNP)
    ident_arr = np.eye(128, dtype=np.float32)

    in_maps = []
    for c in range(N_CORES):
        iA, dA, iB, dB = per_core[c]
        in_maps.append(
            {
                "xlo": xlo,
                "xhi": xhi,
                "wt": wt,
                "bias": bcol,
                "recip": np.ascontiguousarray(recip_pc[c]),
                "idxa": _wrap_idx(iA),
                "dsta": _wrap_dst(dA),
                "idxb": _wrap_idx(iB),
                "dstb": _wrap_dst(dB),
                "iota": iota_arr,
                "ident": ident_arr,
            }
        )

    res = run_bass_kernel_spmd(nc, in_maps, list(range(N_CORES)))
    LAST["results"] = res
    LAST["exec_time_ns"] = res.exec_time_ns

    out_t = np.concatenate([res.results[c]["out"] for c in range(N_CORES)], axis=1)
    cols = out_t.T  # [N_CORES*NPC, 64]: column (c,w,lane) at c*NPC + w*WIN + lane
    result = np.empty((N_NODES, D), dtype=np.float32)
    flat_nodes = binnode.reshape(N_CORES, N_WIN, WIN)
    for c in range(N_CORES):
        for w in range(N_WIN):
            nodes = flat_nodes[c, w]
            valid = nodes >= 0
            result[nodes[valid]] = cols[
                c * NPC + w * WIN : c * NPC + (w + 1) * WIN
            ][valid]
    return result


# revision 25
# speedup vs baseline: 1.0854x; 1.0151x over previous
"""GCNConv mean-aggregation kernel for 8 Trainium2 NeuronCores.

Reference computation:
    msgs   = x[src]                       # [E, D] gather
    summed = segment_sum(msgs, dst, N)    # [N, D]
    deg    = segment_sum(ones, dst, N)    # [N]
    h      = summed / max(deg, 1)
    out    = h @ W.T + b                  # [N, D_OUT]

Strategy (v2 — SWDGE gather tuned from HW microbenchmarks):
  - Shard edges by contiguous dst ranges: core c owns nodes
    [c*6272, (c+1)*6272).  Each core fully reduces its own node range.
  - Edges are grouped into 128-node dst windows (49 per core).  For each
    128-edge subtile we gather x[src] rows from HBM with dma_gather
    (SWDGE).  Rows are 256 B: 64 feats in bf16 + 64 zero pad.  bf16
    halves DMA-ring drain vs f32 rows and runs the PE at full bf16
    rate; the rel-err budget (2e-2) dwarfs bf16 quantization (~4e-3).
  - Gather calls are 32 subtiles (4096 descriptors) rotated over the 4
    SWDGE queues with single_packet=False.  Measured SWDGE descgen is
    ~2.2-2.5 ns/descriptor serialized on GpSimd — the kernel's
    critical path — so descriptor count is minimized: 128-node windows
    (vs 64) cut subtile padding, and degree is NOT computed on device
    (no weight column; the host precomputes 1/max(deg,1) from dst
    alone, pure index preprocessing).
  - A [128e, 128n] bf16 one-hot per subtile (DVE is_equal against an
    iota, 2x DVE rate in 16-bit) feeds  onehot.T @ msgs  into a
    [128, 64] PSUM f32 accumulation chain per window.
  - Normalize with the uploaded per-node reciprocals, PE-transpose
    [128, 64] -> [64, 128], apply W (lhsT = W.T) and bias, and write
    out.T slices ([64, 6272] per core).  Host reassembles/transposes.
  - dma_gather indices are int16, so x is staged into two gather tables
    (src < 32767 and src >= 32767), each with a zero row at index 0
    used by padding edges (contributes 0 to the window sums).
"""

import sys

sys.path.insert(0, "/opt/trn_rl_repo")

import ml_dtypes
import numpy as np

import concourse.bacc as bacc
import concourse.mybir as mybir
import concourse.tile as tile
from concourse.bass_utils import run_bass_kernel_spmd

N_NODES = 50000
N_EDGES = 800000
D = 64
N_CORES = 8
NPC = 6272          # nodes per core (= 49 windows of 128)
WIN = 128           # dst-window width per PSUM accumulation group
N_WIN = NPC // WIN  # 49
SPLIT = 32767       # src < SPLIT -> lo table, else hi table
ROW = 128           # gather row: 64 bf16 feats + 64 bf16 zero pad (256 B)
CHUNK = 32          # subtiles (of 128 edges) per dma_gather call
NQ = 4              # SWDGE queues for parallel gather descriptor work

F32 = mybir.dt.float32
BF16 = mybir.dt.bfloat16
I16 = mybir.dt.int16
BFNP = ml_dtypes.bfloat16

# Results of the most recent run (for test harness inspection).
LAST = {}


def _prep(x, src, dst):
    """Host-side sharding: build bf16 gather tables, per-core padded edge
    streams (int16 gather idx + bf16 dst-rel), per-core reciprocal
    degrees, and per-window subtile budgets (shared across cores; SPMD
    program structure)."""
    x = np.asarray(x, dtype=np.float32)
    src = np.asarray(src, dtype=np.int64)
    dst = np.asarray(dst, dtype=np.int64)

    n_lo = SPLIT
    n_hi = N_NODES - SPLIT
    xlo = np.zeros((n_lo + 1, ROW), dtype=BFNP)
    xlo[1:, :D] = x[:SPLIT].astype(BFNP)
    xhi = np.zeros((n_hi + 1, ROW), dtype=BFNP)
    xhi[1:, :D] = x[SPLIT:].astype(BFNP)

    cls = (src >= SPLIT).astype(np.int64)

    # --- balance nodes across the 392 (core, window) bins ---------------
    # Subtile padding is ceil(max-over-cores/128) per (window, class); an
    # LPT assignment of nodes (weighted by per-class in-degree) makes the
    # per-bin counts nearly equal, collapsing the padding.  The host owns
    # the node->(core, window, lane) permutation and reassembles at the
    # end, so the device never sees node ids.
    import heapq

    a_deg = np.bincount(dst[cls == 0], minlength=N_NODES)
    b_deg = np.bincount(dst[cls == 1], minlength=N_NODES)
    tot_deg = a_deg + b_deg
    n_bins = N_CORES * N_WIN
    order_n = np.argsort(-tot_deg, kind="stable")
    heap = [(0.0, b) for b in range(n_bins)]
    heapq.heapify(heap)
    bin_fill = np.zeros(n_bins, dtype=np.int64)
    bin_a = np.zeros(n_bins, dtype=np.int64)
    bin_b = np.zeros(n_bins, dtype=np.int64)
    node_bin = np.empty(N_NODES, dtype=np.int64)
    node_lane = np.empty(N_NODES, dtype=np.int64)
    wa = 1.0 / max(1, a_deg.sum() // n_bins)
    wb = 1.0 / max(1, b_deg.sum() // n_bins)
    for v in order_n:
        while True:
            load, bn = heapq.heappop(heap)
            if bin_fill[bn] < WIN:
                break
        node_bin[v] = bn
        node_lane[v] = bin_fill[bn]
        bin_fill[bn] += 1
        bin_a[bn] += a_deg[v]
        bin_b[bn] += b_deg[v]
        if bin_fill[bn] < WIN:
            heapq.heappush(heap, (bin_a[bn] * wa + bin_b[bn] * wb, bn))

    deg = np.maximum(tot_deg, 1).astype(np.float32)
    recip_pc = np.ones((N_CORES, WIN, N_WIN), dtype=np.float32)
    for c in range(N_CORES):
        for w in range(N_WIN):
            bn = c * N_WIN + w
            sel = node_bin == bn
            recip_pc[c, node_lane[sel], w] = 1.0 / deg[sel]
    binnode = np.full((n_bins, WIN), -1, dtype=np.int64)
    binnode[node_bin, node_lane] = np.arange(N_NODES)

    key = node_bin[dst] * 2 + cls
    order = np.argsort(key, kind="stable")
    src_s, dst_s = src[order], dst[order]
    lane_s = node_lane[dst_s]

    n_groups = (N_CORES * N_WIN) * 2
    counts = np.bincount(key[order], minlength=n_groups)
    starts = np.zeros(n_groups + 1, dtype=np.int64)
    np.cumsum(counts, out=starts[1:])

    cnt = counts.reshape(N_CORES, N_WIN, 2)
    kA = np.maximum(1, -(-cnt[:, :, 0].max(axis=0) // 128))  # [N_WIN]
    kB = -(-cnt[:, :, 1].max(axis=0) // 128)                  # [N_WIN]
    SA = int(kA.sum())
    SB = int(kB.sum())

    idx_lo = (src_s + 1).astype(np.int16)
    idx_hi = (src_s - SPLIT + 1).astype(np.int16)

    offA = np.zeros(N_WIN + 1, dtype=np.int64)
    np.cumsum(kA, out=offA[1:])
    offB = np.zeros(N_WIN + 1, dtype=np.int64)
    np.cumsum(kB, out=offB[1:])

    per_core = []
    for c in range(N_CORES):
        # pad lanes: idx -1 (SWDGE skips the descriptor entirely — free)
        # and dst_rel 200 (outside iota range, so the one-hot column is
        # all-zero and the stale SBUF lane never reaches the PSUM sums).
        iA = np.zeros(SA * 128, dtype=np.int16)
        dA = np.full(SA * 128, 200.0, dtype=BFNP)
        iB = np.zeros(max(SB, 1) * 128, dtype=np.int16)
        dB = np.full(max(SB, 1) * 128, 200.0, dtype=BFNP)
        for w in range(N_WIN):
            g = (c * N_WIN + w) * 2
            s0, s1 = starts[g], starts[g + 1]
            p0 = int(offA[w]) * 128
            iA[p0 : p0 + (s1 - s0)] = idx_lo[s0:s1]
            dA[p0 : p0 + (s1 - s0)] = lane_s[s0:s1].astype(BFNP)
            s0, s1 = starts[g + 1], starts[g + 2]
            p0 = int(offB[w]) * 128
            iB[p0 : p0 + (s1 - s0)] = idx_hi[s0:s1]
            dB[p0 : p0 + (s1 - s0)] = lane_s[s0:s1].astype(BFNP)
        per_core.append((iA, dA, iB, dB))

    return xlo, xhi, kA, kB, SA, SB, offA, offB, per_core, recip_pc, binnode


def _wrap_idx(idx_flat):
    """int16 stream -> dma_gather layout [128, n/16]: value i at
    [i % 16, i // 16], replicated across the 8 groups of 16 partitions."""
    a = idx_flat.reshape(-1, 16).T
    return np.tile(a, (8, 1)).copy()


def _wrap_dst(d_flat):
    """f32 stream -> [128, S]: subtile s lane e at [e, s]."""
    return np.ascontiguousarray(d_flat.reshape(-1, 128).T)


def _build_program(kA, kB, SA, SB, offA, offB):
    nc = bacc.Bacc(
        "TRN2", target_bir_lowering=False, debug=False, num_swdge_queues=NQ
    )

    t_xlo = nc.dram_tensor("xlo", [SPLIT + 1, ROW], BF16, kind="ExternalInput")
    t_xhi = nc.dram_tensor(
        "xhi", [N_NODES - SPLIT + 1, ROW], BF16, kind="ExternalInput"
    )
    t_wt = nc.dram_tensor("wt", [D, D], F32, kind="ExternalInput")
    t_b = nc.dram_tensor("bias", [D, 1], F32, kind="ExternalInput")
    t_rc = nc.dram_tensor("recip", [WIN, N_WIN], F32, kind="ExternalInput")
    t_ia = nc.dram_tensor("idxa", [128, SA * 8], I16, kind="ExternalInput")
    t_da = nc.dram_tensor("dsta", [128, SA], BF16, kind="ExternalInput")
    SBp = max(SB, 1)
    t_ib = nc.dram_tensor("idxb", [128, SBp * 8], I16, kind="ExternalInput")
    t_db = nc.dram_tensor("dstb", [128, SBp], BF16, kind="ExternalInput")
    t_iota = nc.dram_tensor("iota", [128, WIN], BF16, kind="ExternalInput")
    t_id = nc.dram_tensor("ident", [128, 128], F32, kind="ExternalInput")
    t_out = nc.dram_tensor("out", [D, NPC], F32, kind="ExternalOutput")

    def _calls(S):
        # ramp: small leading calls so consumers start early and the cold
        # first drains block less; small trailing calls to shrink the tail
        sizes = []
        rem = S
        for sz in (8, 8, 16, 16):
            if rem <= sz:
                break
            sizes.append(sz)
            rem -= sz
        while rem > CHUNK:
            sizes.append(CHUNK)
            rem -= CHUNK
        if rem > CHUNK // 2:
            sizes += [rem - rem // 2, rem // 2]
        else:
            sizes.append(rem)
        calls = []
        p = 0
        for sz in sizes:
            calls.append((p, sz))
            p += sz
        return calls

    callsA = _calls(SA)
    callsB = _calls(SB)

    with tile.TileContext(nc) as tc:
        with (
            tc.tile_pool(name="const", bufs=1) as cpool,
            tc.tile_pool(name="idx", bufs=1) as ipool,
            tc.tile_pool(name="msgsa", bufs=4) as mpa,
            tc.tile_pool(name="msgsb", bufs=3) as mpb,
            tc.tile_pool(name="oha", bufs=4) as opa,
            tc.tile_pool(name="ohb", bufs=3) as opb,
            tc.tile_pool(name="norm", bufs=6) as npool,
            tc.tile_pool(name="hpo", bufs=4) as hpool,
            tc.tile_pool(name="psacc", bufs=4, space="PSUM") as ps_acc,
            tc.tile_pool(name="pstr", bufs=2, space="PSUM") as ps_tr,
            tc.tile_pool(name="psz", bufs=2, space="PSUM") as ps_z,
        ):
            ident = cpool.tile([128, 128], F32)
            nc.sync.dma_start(out=ident[:], in_=t_id[:])
            wt_sb = cpool.tile([D, D], F32)
            nc.sync.dma_start(out=wt_sb[:], in_=t_wt[:])
            b_sb = cpool.tile([D, 1], F32)
            nc.sync.dma_start(out=b_sb[:], in_=t_b[:])
            rc_sb = cpool.tile([WIN, N_WIN], F32)
            nc.sync.dma_start(out=rc_sb[:], in_=t_rc[:])
            iota_f = cpool.tile([128, WIN], BF16)
            nc.sync.dma_start(out=iota_f[:], in_=t_iota[:])

            ia_sb = ipool.tile([128, SA * 8], I16)
            da_sb = ipool.tile([128, SA], BF16)
            nc.sync.dma_start(out=da_sb[:], in_=t_da[:])
            ib_sb = ipool.tile([128, SBp * 8], I16)
            db_sb = ipool.tile([128, SBp], BF16)
            nc.sync.dma_start(out=db_sb[:], in_=t_db[:])

            out_sb = cpool.tile([D, NPC], F32)

            chunk_tiles = {0: [], 1: []}
            call_no = [0]

            def emit_chunk(st, k):
                if st == 0:
                    pos, nsub = callsA[k]
                    mp, op, tsrc, isb, dsb = mpa, opa, t_xlo, ia_sb, da_sb
                else:
                    pos, nsub = callsB[k]
                    mp, op, tsrc, isb, dsb = mpb, opb, t_xhi, ib_sb, db_sb
                # just-in-time idx slice load: spreads the 2.7 MB of index
                # tables across the run instead of jamming the DMA engines
                # (and gather-ring drains) at startup
                t_i = t_ia if st == 0 else t_ib
                nc.sync.dma_start(
                    out=isb[:, pos * 8 : pos * 8 + nsub * 8],
                    in_=t_i[:, pos * 8 : pos * 8 + nsub * 8],
                )
                msgs = mp.tile([128, CHUNK, ROW], BF16)
                nidx = nsub * 128
                # single_packet=False: one packet per descriptor (the default
                # single-packet mode wedges the SDMA engine beyond ~64
                # descs/lane).  Rotating queue_num spreads ring drain over
                # the 4 SWDGE queues.
                nc.gpsimd.dma_gather(
                    msgs[:, :nsub, :],
                    tsrc[:],
                    isb[:, pos * 8 : pos * 8 + nsub * 8],
                    nidx,
                    nidx,
                    ROW,
                    single_packet=False,
                    queue_num=call_no[0] % NQ,
                )
                call_no[0] += 1
                # chunked one-hot: f32 iota vs broadcast f32 dst-rel, bf16
                # out for the full-rate bf16 matmul.  (Per-subtile
                # tensor_scalar variants lose: ~578 ns fixed cost per DVE
                # instruction.)
                oh = op.tile([128, CHUNK * WIN], BF16)
                dst_b = (
                    dsb[:, pos : pos + nsub]
                    .unsqueeze(2)
                    .to_broadcast([128, nsub, WIN])
                )
                nc.vector.tensor_tensor(
                    out=oh[:, : nsub * WIN].rearrange("p (s w) -> p s w", w=WIN),
                    in0=iota_f[:].unsqueeze(1).to_broadcast([128, nsub, WIN]),
                    in1=dst_b,
                    op=mybir.AluOpType.is_equal,
                )
                chunk_tiles[st].append((msgs, oh))

            cursor = [0, 0]
            call_of = {}
            for st, calls in ((0, callsA), (1, callsB)):
                for k, (pos, nsub) in enumerate(calls):
                    for s in range(pos, pos + nsub):
                        call_of[(st, s)] = (k, s - pos)

            def tiles_for(st, s):
                k, col = call_of[(st, s)]
                while cursor[st] <= k:
                    emit_chunk(st, cursor[st])
                    cursor[st] += 1
                msgs, oh = chunk_tiles[st][k]
                return msgs, oh, col

            for w in range(N_WIN):
                subs = [(0, int(offA[w]) + j) for j in range(int(kA[w]))]
                subs += [(1, int(offB[w]) + j) for j in range(int(kB[w]))]
                ps = ps_acc.tile([WIN, D], F32)
                for j, (st, s) in enumerate(subs):
                    msgs, oh, col = tiles_for(st, s)
                    nc.tensor.matmul(
                        out=ps[:],
                        lhsT=oh[:, col * WIN : (col + 1) * WIN],
                        rhs=msgs[:, col, :D],
                        start=(j == 0),
                        stop=(j == len(subs) - 1),
                    )
                h_w = npool.tile([WIN, D], F32)
                nc.vector.tensor_scalar_mul(h_w[:], ps[:], rc_sb[:, w : w + 1])
                pst = ps_tr.tile([D, WIN], F32)
                nc.tensor.transpose(out=pst[:], in_=h_w[:], identity=ident[:])
                ht = hpool.tile([D, WIN], F32)
                nc.vector.tensor_copy(out=ht[:], in_=pst[:])
                z = ps_z.tile([D, WIN], F32)
                nc.tensor.matmul(
                    out=z[:], lhsT=wt_sb[:], rhs=ht[:], start=True, stop=True
                )
                nc.vector.tensor_scalar_add(
                    out_sb[:, w * WIN : (w + 1) * WIN], z[:], b_sb[:]
                )
                if (w + 1) % 8 == 0 or w == N_WIN - 1:
                    w0 = (w // 8) * 8
                    nc.sync.dma_start(
                        out=t_out[:, w0 * WIN : (w + 1) * WIN],
                        in_=out_sb[:, w0 * WIN : (w + 1) * WIN],
                    )

    nc.compile()
    return nc


def kernel(x, src, dst, W, b):
    x = np.asarray(x, dtype=np.float32)
    W = np.asarray(W, dtype=np.float32)
    b = np.asarray(b, dtype=np.float32)

    (xlo, xhi, kA, kB, SA, SB, offA, offB, per_core, recip_pc,
     binnode) = _prep(x, src, dst)
    print(f"kernel: SA={SA} SB={SB} descs={(SA+SB)*128}", file=sys.stderr)
    nc = _build_program(kA, kB, SA, SB, offA, offB)

    wt = np.ascontiguousarray(W.T)
    bcol = np.ascontiguousarray(b.reshape(D, 1))
    iota_arr = np.tile(
        np.arange(WIN, dtype=np.float32)[None, :], (128, 1)
    ).astype(BFNP)
    ident_arr = np.eye(128, dtype=np.float32)

    in_maps = []
    for c in range(N_CORES):
        iA, dA, iB, dB = per_core[c]
        in_maps.append(
            {
                "xlo": xlo,
                "xhi": xhi,
                "wt": wt,
                "bias": bcol,
                "recip": np.ascontiguousarray(recip_pc[c]),
                "idxa": _wrap_idx(iA),
                "dsta": _wrap_dst(dA),
                "idxb": _wrap_idx(iB),
                "dstb": _wrap_dst(dB),
                "iota": iota_arr,
                "ident": ident_arr,
            }
        )

    res = run_bass_kernel_spmd(nc, in_maps, list(range(N_CORES)))
    LAST["results"] = res
    LAST["exec_time_ns"] = res.exec_time_ns

    out_t = np.concatenate([res.results[c]["out"] for c in range(N_CORES)], axis=1)
    cols = out_t.T  # [N_CORES*NPC, 64]: column (c,w,lane) at c*NPC + w*WIN + lane
    result = np.empty((N_NODES, D), dtype=np.float32)
    flat_nodes = binnode.reshape(N_CORES, N_WIN, WIN)
    for c in range(N_CORES):
        for w in range(N_WIN):
            nodes = flat_nodes[c, w]
            valid = nodes >= 0
            result[nodes[valid]] = cols[
                c * NPC + w * WIN : c * NPC + (w + 1) * WIN
            ][valid]
    return result


# revision 26
# speedup vs baseline: 1.1326x; 1.0435x over previous
"""GCNConv mean-aggregation kernel for 8 Trainium2 NeuronCores.

Reference computation:
    msgs   = x[src]                       # [E, D] gather
    summed = segment_sum(msgs, dst, N)    # [N, D]
    deg    = segment_sum(ones, dst, N)    # [N]
    h      = summed / max(deg, 1)
    out    = h @ W.T + b                  # [N, D_OUT]

Strategy (v2 — SWDGE gather tuned from HW microbenchmarks):
  - Shard edges by contiguous dst ranges: core c owns nodes
    [c*6272, (c+1)*6272).  Each core fully reduces its own node range.
  - Edges are grouped into 128-node dst windows (49 per core).  For each
    128-edge subtile we gather x[src] rows from HBM with dma_gather
    (SWDGE).  Rows are 256 B: 64 feats in bf16 + 64 zero pad.  bf16
    halves DMA-ring drain vs f32 rows and runs the PE at full bf16
    rate; the rel-err budget (2e-2) dwarfs bf16 quantization (~4e-3).
  - Gather calls are 32 subtiles (4096 descriptors) rotated over the 4
    SWDGE queues with single_packet=False.  Measured SWDGE descgen is
    ~2.2-2.5 ns/descriptor serialized on GpSimd — the kernel's
    critical path — so descriptor count is minimized: 128-node windows
    (vs 64) cut subtile padding, and degree is NOT computed on device
    (no weight column; the host precomputes 1/max(deg,1) from dst
    alone, pure index preprocessing).
  - A [128e, 128n] bf16 one-hot per subtile (DVE is_equal against an
    iota, 2x DVE rate in 16-bit) feeds  onehot.T @ msgs  into a
    [128, 64] PSUM f32 accumulation chain per window.
  - Normalize with the uploaded per-node reciprocals, PE-transpose
    [128, 64] -> [64, 128], apply W (lhsT = W.T) and bias, and write
    out.T slices ([64, 6272] per core).  Host reassembles/transposes.
  - dma_gather indices are int16, so x is staged into two gather tables
    (src < 32767 and src >= 32767), each with a zero row at index 0
    used by padding edges (contributes 0 to the window sums).
"""

import sys

sys.path.insert(0, "/opt/trn_rl_repo")

import ml_dtypes
import numpy as np

import concourse.bacc as bacc
import concourse.mybir as mybir
import concourse.tile as tile
from concourse.bass_utils import run_bass_kernel_spmd

N_NODES = 50000
N_EDGES = 800000
D = 64
N_CORES = 8
NPC = 6272          # nodes per core (= 49 windows of 128)
WIN = 128           # dst-window width per PSUM accumulation group
N_WIN = NPC // WIN  # 49
SPLIT = 32767       # src < SPLIT -> lo table, else hi table
ROW = 128           # gather row: 64 bf16 feats + 64 bf16 zero pad (256 B)
CHUNK = 32          # subtiles (of 128 edges) per dma_gather call
NQ = 4              # SWDGE queues for parallel gather descriptor work

F32 = mybir.dt.float32
BF16 = mybir.dt.bfloat16
I16 = mybir.dt.int16
BFNP = ml_dtypes.bfloat16

# Results of the most recent run (for test harness inspection).
LAST = {}


def _prep(x, src, dst):
    """Host-side sharding: build bf16 gather tables, per-core padded edge
    streams (int16 gather idx + bf16 dst-rel), per-core reciprocal
    degrees, and per-window subtile budgets (shared across cores; SPMD
    program structure)."""
    x = np.asarray(x, dtype=np.float32)
    src = np.asarray(src, dtype=np.int64)
    dst = np.asarray(dst, dtype=np.int64)

    n_lo = SPLIT
    n_hi = N_NODES - SPLIT
    xlo = np.zeros((n_lo + 1, ROW), dtype=BFNP)
    xlo[1:, :D] = x[:SPLIT].astype(BFNP)
    xhi = np.zeros((n_hi + 1, ROW), dtype=BFNP)
    xhi[1:, :D] = x[SPLIT:].astype(BFNP)

    cls = (src >= SPLIT).astype(np.int64)

    # --- balance nodes across the 392 (core, window) bins ---------------
    # Subtile padding is ceil(max-over-cores/128) per (window, class); an
    # LPT assignment of nodes (weighted by per-class in-degree) makes the
    # per-bin counts nearly equal, collapsing the padding.  The host owns
    # the node->(core, window, lane) permutation and reassembles at the
    # end, so the device never sees node ids.
    import heapq

    a_deg = np.bincount(dst[cls == 0], minlength=N_NODES)
    b_deg = np.bincount(dst[cls == 1], minlength=N_NODES)
    tot_deg = a_deg + b_deg
    n_bins = N_CORES * N_WIN
    order_n = np.argsort(-tot_deg, kind="stable")
    heap = [(0.0, b) for b in range(n_bins)]
    heapq.heapify(heap)
    bin_fill = np.zeros(n_bins, dtype=np.int64)
    bin_a = np.zeros(n_bins, dtype=np.int64)
    bin_b = np.zeros(n_bins, dtype=np.int64)
    node_bin = np.empty(N_NODES, dtype=np.int64)
    node_lane = np.empty(N_NODES, dtype=np.int64)
    wa = 1.0 / max(1, a_deg.sum() // n_bins)
    wb = 1.0 / max(1, b_deg.sum() // n_bins)
    for v in order_n:
        while True:
            load, bn = heapq.heappop(heap)
            if bin_fill[bn] < WIN:
                break
        node_bin[v] = bn
        node_lane[v] = bin_fill[bn]
        bin_fill[bn] += 1
        bin_a[bn] += a_deg[v]
        bin_b[bn] += b_deg[v]
        if bin_fill[bn] < WIN:
            heapq.heappush(heap, (bin_a[bn] * wa + bin_b[bn] * wb, bn))

    deg = np.maximum(tot_deg, 1).astype(np.float32)
    recip_pc = np.ones((N_CORES, WIN, N_WIN), dtype=np.float32)
    for c in range(N_CORES):
        for w in range(N_WIN):
            bn = c * N_WIN + w
            sel = node_bin == bn
            recip_pc[c, node_lane[sel], w] = 1.0 / deg[sel]
    binnode = np.full((n_bins, WIN), -1, dtype=np.int64)
    binnode[node_bin, node_lane] = np.arange(N_NODES)

    key = node_bin[dst] * 2 + cls
    order = np.argsort(key, kind="stable")
    src_s, dst_s = src[order], dst[order]
    lane_s = node_lane[dst_s]

    n_groups = (N_CORES * N_WIN) * 2
    counts = np.bincount(key[order], minlength=n_groups)
    starts = np.zeros(n_groups + 1, dtype=np.int64)
    np.cumsum(counts, out=starts[1:])

    cnt = counts.reshape(N_CORES, N_WIN, 2)
    kA = np.maximum(1, -(-cnt[:, :, 0].max(axis=0) // 128))  # [N_WIN]
    kB = -(-cnt[:, :, 1].max(axis=0) // 128)                  # [N_WIN]
    SA = int(kA.sum())
    SB = int(kB.sum())

    idx_lo = (src_s + 1).astype(np.int16)
    idx_hi = (src_s - SPLIT + 1).astype(np.int16)

    offA = np.zeros(N_WIN + 1, dtype=np.int64)
    np.cumsum(kA, out=offA[1:])
    offB = np.zeros(N_WIN + 1, dtype=np.int64)
    np.cumsum(kB, out=offB[1:])

    per_core = []
    for c in range(N_CORES):
        # pad lanes: idx -1 (SWDGE skips the descriptor entirely — free)
        # and dst_rel 200 (outside iota range, so the one-hot column is
        # all-zero and the stale SBUF lane never reaches the PSUM sums).
        iA = np.zeros(SA * 128, dtype=np.int16)
        dA = np.full(SA * 128, 200.0, dtype=BFNP)
        iB = np.zeros(max(SB, 1) * 128, dtype=np.int16)
        dB = np.full(max(SB, 1) * 128, 200.0, dtype=BFNP)
        for w in range(N_WIN):
            g = (c * N_WIN + w) * 2
            s0, s1 = starts[g], starts[g + 1]
            p0 = int(offA[w]) * 128
            iA[p0 : p0 + (s1 - s0)] = idx_lo[s0:s1]
            dA[p0 : p0 + (s1 - s0)] = lane_s[s0:s1].astype(BFNP)
            s0, s1 = starts[g + 1], starts[g + 2]
            p0 = int(offB[w]) * 128
            iB[p0 : p0 + (s1 - s0)] = idx_hi[s0:s1]
            dB[p0 : p0 + (s1 - s0)] = lane_s[s0:s1].astype(BFNP)
        per_core.append((iA, dA, iB, dB))

    return xlo, xhi, kA, kB, SA, SB, offA, offB, per_core, recip_pc, binnode


def _wrap_idx(idx_flat):
    """int16 stream -> dma_gather layout [128, n/16]: value i at
    [i % 16, i // 16], replicated across the 8 groups of 16 partitions."""
    a = idx_flat.reshape(-1, 16).T
    return np.tile(a, (8, 1)).copy()


def _wrap_dst(d_flat):
    """f32 stream -> [128, S]: subtile s lane e at [e, s]."""
    return np.ascontiguousarray(d_flat.reshape(-1, 128).T)


def _build_program(kA, kB, SA, SB, offA, offB):
    nc = bacc.Bacc(
        "TRN2", target_bir_lowering=False, debug=False, num_swdge_queues=NQ
    )

    t_xlo = nc.dram_tensor("xlo", [SPLIT + 1, ROW], BF16, kind="ExternalInput")
    t_xhi = nc.dram_tensor(
        "xhi", [N_NODES - SPLIT + 1, ROW], BF16, kind="ExternalInput"
    )
    t_wt = nc.dram_tensor("wt", [D, D], F32, kind="ExternalInput")
    t_b = nc.dram_tensor("bias", [D, 1], F32, kind="ExternalInput")
    t_rc = nc.dram_tensor("recip", [WIN, N_WIN], F32, kind="ExternalInput")
    t_ia = nc.dram_tensor("idxa", [128, SA * 8], I16, kind="ExternalInput")
    t_da = nc.dram_tensor("dsta", [128, SA], BF16, kind="ExternalInput")
    SBp = max(SB, 1)
    t_ib = nc.dram_tensor("idxb", [128, SBp * 8], I16, kind="ExternalInput")
    t_db = nc.dram_tensor("dstb", [128, SBp], BF16, kind="ExternalInput")
    t_iota = nc.dram_tensor("iota", [128, WIN], BF16, kind="ExternalInput")
    t_id = nc.dram_tensor("ident", [128, 128], F32, kind="ExternalInput")
    t_out = nc.dram_tensor("out", [D, NPC], F32, kind="ExternalOutput")

    def _calls(S):
        # ramp: small leading calls so consumers start early and the cold
        # first drains block less; small trailing calls to shrink the tail
        sizes = []
        rem = S
        for sz in (8, 8, 16, 16):
            if rem <= sz:
                break
            sizes.append(sz)
            rem -= sz
        while rem > CHUNK:
            sizes.append(CHUNK)
            rem -= CHUNK
        if rem > CHUNK // 2:
            sizes += [rem - rem // 2, rem // 2]
        else:
            sizes.append(rem)
        calls = []
        p = 0
        for sz in sizes:
            calls.append((p, sz))
            p += sz
        return calls

    callsA = _calls(SA)
    callsB = _calls(SB)

    with tile.TileContext(nc) as tc:
        with (
            tc.tile_pool(name="const", bufs=1) as cpool,
            tc.tile_pool(name="idx", bufs=1) as ipool,
            tc.tile_pool(name="msgsa", bufs=5) as mpa,
            tc.tile_pool(name="msgsb", bufs=4) as mpb,
            tc.tile_pool(name="oha", bufs=5) as opa,
            tc.tile_pool(name="ohb", bufs=4) as opb,
            tc.tile_pool(name="norm", bufs=6) as npool,
            tc.tile_pool(name="hpo", bufs=4) as hpool,
            tc.tile_pool(name="psacc", bufs=4, space="PSUM") as ps_acc,
            tc.tile_pool(name="pstr", bufs=2, space="PSUM") as ps_tr,
            tc.tile_pool(name="psz", bufs=2, space="PSUM") as ps_z,
        ):
            ident = cpool.tile([128, 128], F32)
            nc.sync.dma_start(out=ident[:], in_=t_id[:])
            wt_sb = cpool.tile([D, D], F32)
            nc.sync.dma_start(out=wt_sb[:], in_=t_wt[:])
            b_sb = cpool.tile([D, 1], F32)
            nc.sync.dma_start(out=b_sb[:], in_=t_b[:])
            rc_sb = cpool.tile([WIN, N_WIN], F32)
            nc.sync.dma_start(out=rc_sb[:], in_=t_rc[:])
            iota_f = cpool.tile([128, WIN], BF16)
            nc.sync.dma_start(out=iota_f[:], in_=t_iota[:])

            ia_sb = ipool.tile([128, SA * 8], I16)
            da_sb = ipool.tile([128, SA], BF16)
            nc.sync.dma_start(out=da_sb[:], in_=t_da[:])
            ib_sb = ipool.tile([128, SBp * 8], I16)
            db_sb = ipool.tile([128, SBp], BF16)
            nc.sync.dma_start(out=db_sb[:], in_=t_db[:])

            out_sb = cpool.tile([D, NPC], F32)

            chunk_tiles = {0: [], 1: []}
            call_no = [0]

            def emit_chunk(st, k):
                if st == 0:
                    pos, nsub = callsA[k]
                    mp, op, tsrc, isb, dsb = mpa, opa, t_xlo, ia_sb, da_sb
                else:
                    pos, nsub = callsB[k]
                    mp, op, tsrc, isb, dsb = mpb, opb, t_xhi, ib_sb, db_sb
                # just-in-time idx slice load: spreads the 2.7 MB of index
                # tables across the run instead of jamming the DMA engines
                # (and gather-ring drains) at startup
                t_i = t_ia if st == 0 else t_ib
                nc.sync.dma_start(
                    out=isb[:, pos * 8 : pos * 8 + nsub * 8],
                    in_=t_i[:, pos * 8 : pos * 8 + nsub * 8],
                )
                msgs = mp.tile([128, CHUNK, ROW], BF16)
                nidx = nsub * 128
                # single_packet=False: one packet per descriptor (the default
                # single-packet mode wedges the SDMA engine beyond ~64
                # descs/lane).  Rotating queue_num spreads ring drain over
                # the 4 SWDGE queues.
                nc.gpsimd.dma_gather(
                    msgs[:, :nsub, :],
                    tsrc[:],
                    isb[:, pos * 8 : pos * 8 + nsub * 8],
                    nidx,
                    nidx,
                    ROW,
                    single_packet=False,
                    queue_num=call_no[0] % NQ,
                )
                call_no[0] += 1
                # chunked one-hot: f32 iota vs broadcast f32 dst-rel, bf16
                # out for the full-rate bf16 matmul.  (Per-subtile
                # tensor_scalar variants lose: ~578 ns fixed cost per DVE
                # instruction.)
                oh = op.tile([128, CHUNK * WIN], BF16)
                dst_b = (
                    dsb[:, pos : pos + nsub]
                    .unsqueeze(2)
                    .to_broadcast([128, nsub, WIN])
                )
                nc.vector.tensor_tensor(
                    out=oh[:, : nsub * WIN].rearrange("p (s w) -> p s w", w=WIN),
                    in0=iota_f[:].unsqueeze(1).to_broadcast([128, nsub, WIN]),
                    in1=dst_b,
                    op=mybir.AluOpType.is_equal,
                )
                chunk_tiles[st].append((msgs, oh))

            cursor = [0, 0]
            call_of = {}
            for st, calls in ((0, callsA), (1, callsB)):
                for k, (pos, nsub) in enumerate(calls):
                    for s in range(pos, pos + nsub):
                        call_of[(st, s)] = (k, s - pos)

            def tiles_for(st, s):
                k, col = call_of[(st, s)]
                while cursor[st] <= k:
                    emit_chunk(st, cursor[st])
                    cursor[st] += 1
                msgs, oh = chunk_tiles[st][k]
                return msgs, oh, col

            for w in range(N_WIN):
                subs = [(0, int(offA[w]) + j) for j in range(int(kA[w]))]
                subs += [(1, int(offB[w]) + j) for j in range(int(kB[w]))]
                ps = ps_acc.tile([WIN, D], F32)
                for j, (st, s) in enumerate(subs):
                    msgs, oh, col = tiles_for(st, s)
                    nc.tensor.matmul(
                        out=ps[:],
                        lhsT=oh[:, col * WIN : (col + 1) * WIN],
                        rhs=msgs[:, col, :D],
                        start=(j == 0),
                        stop=(j == len(subs) - 1),
                    )
                h_w = npool.tile([WIN, D], F32)
                nc.vector.tensor_scalar_mul(h_w[:], ps[:], rc_sb[:, w : w + 1])
                pst = ps_tr.tile([D, WIN], F32)
                nc.tensor.transpose(out=pst[:], in_=h_w[:], identity=ident[:])
                ht = hpool.tile([D, WIN], F32)
                nc.vector.tensor_copy(out=ht[:], in_=pst[:])
                z = ps_z.tile([D, WIN], F32)
                nc.tensor.matmul(
                    out=z[:], lhsT=wt_sb[:], rhs=ht[:], start=True, stop=True
                )
                nc.vector.tensor_scalar_add(
                    out_sb[:, w * WIN : (w + 1) * WIN], z[:], b_sb[:]
                )
                if (w + 1) % 8 == 0 or w == N_WIN - 1:
                    w0 = (w // 8) * 8
                    nc.sync.dma_start(
                        out=t_out[:, w0 * WIN : (w + 1) * WIN],
                        in_=out_sb[:, w0 * WIN : (w + 1) * WIN],
                    )

    nc.compile()
    return nc


def kernel(x, src, dst, W, b):
    x = np.asarray(x, dtype=np.float32)
    W = np.asarray(W, dtype=np.float32)
    b = np.asarray(b, dtype=np.float32)

    (xlo, xhi, kA, kB, SA, SB, offA, offB, per_core, recip_pc,
     binnode) = _prep(x, src, dst)
    print(f"kernel: SA={SA} SB={SB} descs={(SA+SB)*128}", file=sys.stderr)
    nc = _build_program(kA, kB, SA, SB, offA, offB)

    wt = np.ascontiguousarray(W.T)
    bcol = np.ascontiguousarray(b.reshape(D, 1))
    iota_arr = np.tile(
        np.arange(WIN, dtype=np.float32)[None, :], (128, 1)
    ).astype(BFNP)
    ident_arr = np.eye(128, dtype=np.float32)

    in_maps = []
    for c in range(N_CORES):
        iA, dA, iB, dB = per_core[c]
        in_maps.append(
            {
                "xlo": xlo,
                "xhi": xhi,
                "wt": wt,
                "bias": bcol,
                "recip": np.ascontiguousarray(recip_pc[c]),
                "idxa": _wrap_idx(iA),
                "dsta": _wrap_dst(dA),
                "idxb": _wrap_idx(iB),
                "dstb": _wrap_dst(dB),
                "iota": iota_arr,
                "ident": ident_arr,
            }
        )

    res = run_bass_kernel_spmd(nc, in_maps, list(range(N_CORES)))
    LAST["results"] = res
    LAST["exec_time_ns"] = res.exec_time_ns

    out_t = np.concatenate([res.results[c]["out"] for c in range(N_CORES)], axis=1)
    cols = out_t.T  # [N_CORES*NPC, 64]: column (c,w,lane) at c*NPC + w*WIN + lane
    result = np.empty((N_NODES, D), dtype=np.float32)
    flat_nodes = binnode.reshape(N_CORES, N_WIN, WIN)
    for c in range(N_CORES):
        for w in range(N_WIN):
            nodes = flat_nodes[c, w]
            valid = nodes >= 0
            result[nodes[valid]] = cols[
                c * NPC + w * WIN : c * NPC + (w + 1) * WIN
            ][valid]
    return result
